# revision 6
# baseline (speedup 1.0000x reference)
"""Trainium2 Bass kernel for nn_MetaBaseline (global-cosine + DN4 few-shot scoring).

Math (per episode b):
  global: logits[q,k] = <qmean_hat, bmean_hat>          (means over the 5x5 spatial grid)
  DN4:    sim[q,p,k,l] = <q_patch[q,p], s_col_hat[k,l]>  -> sum of top-neighbor_k over l,
          summed over p, / neighbor_k
  out = r0 * logits + r1 * dn4

Device strategy (data-parallel, 8 episodes per NeuronCore):
  - the global-cosine branch is 0.3% of the FLOPs and runs on the host
    (means + one small einsum); the device computes only the DN4 branch.
  - host pre-normalizes both sides: s_hat packed [128, nd*125] bf16 per
    episode and q_hat packed [128, nd*1920] (qp-major, zero-padded from 1875)
    per episode in fp8 e4m3 scaled by 16 (halves the dominant HBM stream and
    makes each episode one contiguous 9600B-per-partition DMA; the x16 rides
    through the whole DN4 branch and is divided out on the host).
  - ALL input DMAs ride ONE queue in exact consumption order (the DMA HW runs
    every outstanding transfer concurrently with fair wire sharing, so queue
    FIFO order is the only effective prioritization).
  - PE: sim[qp, 0:125] = q_hat^T @ s_hat as 15 qp-tiles x 5 k-tiles of
    [128,128]x[128,125] fp8xbf16 matmuls; four qp-tiles share one fp32 PSUM
    bank [128,500].  The PE pair stream (LDWEIGHTS ~56ns / MATMUL ~60ns,
    overlapped via the 64-deep reorder window) is the kernel bottleneck.
  - the PE HAM clock-gate holds the PE at 1.2GHz until ~3.4us of sustained
    activity: ~4us of dummy [128,128]x[128,500] matmuls run while episode
    0's data streams in, so real matmuls start at 2.4GHz.
  - top-k: a custom paged DVE op (registered into the per-NEFF uop table)
    mirrors stock MAX8's swap-flop insertion chain but loops per 25-element
    page via SUB_DIM_DONE: seed(8) + steady(17) + a single drain cycle whose
    ALU chain reads swap0 (the largest) and ADDs swaps 1..k-1 as the element
    flows out -- emitting sum-of-top-k directly, one value per page, ~28ns
    per (query-patch, class) group vs ~86ns for one stock MAX8.  It reads
    the fp32 sims straight from PSUM (no ACT copy stage at all).
  - per episode the 4 paged-op results land in a [128, 75] fp32 draw tile,
    DMA'd to the host; the host does the tiny patch->query aggregation
    einsum (0.3% of FLOPs) and the r-weighted combine.
"""
import numpy as np
import ml_dtypes

N_CORES = 8
B, WAY, SHOT, D, H, W = 64, 5, 1, 640, 5, 5
NQ = 75
HW = H * W                 # 25
QP = NQ * HW               # 1875 query patches per episode
NT = 15                    # qp tiles of 128
QP_PAD = NT * 128          # 1920
ND = D // 128              # 5 contraction tiles
EPC = B // N_CORES         # 8 episodes per core
SCOLS = WAY * HW           # 125
GEPS = 1e-12               # eps of the global-cosine branch (torch F.normalize)
QSCALE = 16.0              # fp8 pre-scale for q_hat
HEAD = 384                 # episode-0 head columns (tiles 0-2)
C1 = 1280                  # episode-0 second chunk boundary

_CACHE = {}
_LAST_IN_MAPS = None


# ---------------------------------------------------------------------------
# Custom paged DVE op: per 25-element page, sum of the k largest values.
# ---------------------------------------------------------------------------
_STEADY, _DRAIN, _SEED0L = 8, 9, 10
_DVE_OPS = {}


def _paged_topk_sum_op(k: int):
    """Register (once) and return the paged top-k-sum DveOp for this k."""
    if k in _DVE_OPS:
        return _DVE_OPS[k]
    from concourse.dve_ops import (
        DveOp,
        OPS,
        CUSTOM_DVE_SPECS,
        _CUSTOM_DVE_ROW_BASE,
        _SUB_OPCODE_FOR_NAME,
    )
    from concourse.dve_spec import Spec, Src0
    from concourse.dve_uop import (
        ENABLE,
        AluInp,
        AluOp,
        DveOpSpec,
        InpSel,
        OutSel,
        Trigger,
        UopConfig,
    )

    def seed(j, nxt):
        u = UopConfig()
        u.enable_input(InpSel.SRC_0, 0)
        u.require_inp0 = ENABLE
        u.repeat_count = 1
        u.trigger = (Trigger.SRC_TENSOR_DONE, Trigger.COUNT, Trigger.NONE)
        u.next_uop = (_DRAIN, nxt, 0)
        for s in range(j):
            u.datapath_config[s].enable_alu(
                AluOp.MIN, AluInp.CURR_SWAP_OUT, AluInp.PREV_ALU_OUT)
            u.datapath_config[s].swap_enable = ENABLE
        u.datapath_config[j].pass_through_alu()
        u.datapath_config[j].swap_enable = ENABLE
        return u

    def steady():
        u = UopConfig()
        u.enable_input(InpSel.SRC_0, 0)
        u.require_inp0 = ENABLE
        u.trigger = (Trigger.SUB_DIM_DONE, Trigger.SRC_TENSOR_DONE, Trigger.NONE)
        u.next_uop = (_DRAIN, _DRAIN, 0)
        for s in range(8):
            u.datapath_config[s].enable_alu(
                AluOp.MIN, AluInp.PREV_ALU_OUT, AluInp.CURR_SWAP_OUT)
            u.datapath_config[s].swap_enable = ENABLE
        return u

    def drain():
        u = UopConfig()
        u.repeat_count = 1
        u.trigger = (Trigger.SRC_TENSOR_DONE, Trigger.COUNT, Trigger.NONE)
        u.next_uop = (0, _SEED0L, 0)
        u.datapath_config[0].enable_alu(
            AluOp.BYPASS, AluInp.CURR_SWAP_OUT, AluInp.CURR_SWAP_OUT)
        for s in range(1, k):
            u.datapath_config[s].enable_alu(
                AluOp.ADD, AluInp.PREV_ALU_OUT, AluInp.CURR_SWAP_OUT)
        for s in range(k, 8):
            u.datapath_config[s].pass_through_alu()
        u.enable_output(OutSel.ALU_OUT)
        return u

    uops = [seed(j, j + 1) for j in range(8)]
    uops[7].next_uop = (_DRAIN, _STEADY, 0)
    uops.append(steady())
    uops.append(drain())
    uops.append(seed(0, 1))  # loop re-entry (next_uop 0 means IDLE)

    class RawDveOp(DveOp):
        def __init__(self, name, raw_uops):
            object.__setattr__(self, "name", name)
            object.__setattr__(self, "spec", Spec(body=Src0))
            object.__setattr__(self, "subdim", True)
            object.__setattr__(self, "uops_sha", {})
            object.__setattr__(self, "perf_en", {})
            object.__setattr__(self, "_uops", raw_uops)

        def compile(self, ver):
            from concourse.dve_ops import get_dve_sub_opcode
            return DveOpSpec(
                name=self.name,
                opcode=get_dve_sub_opcode(self.name),
                uops=self._uops,
                rd1_en=False,
            )

    name = f"MAX8_PAGED_SUM{k}_ANT"
    op = RawDveOp(name, uops)
    if name not in _SUB_OPCODE_FOR_NAME:
        OPS.append(op)
        row = _CUSTOM_DVE_ROW_BASE + len(OPS) - 1
        assert row < 0x20, "custom-DVE row overflow"
        _SUB_OPCODE_FOR_NAME[name] = row
        CUSTOM_DVE_SPECS[name] = op.spec
    _DVE_OPS[k] = op
    return op


def _build(k: int):
    """Build + compile the SPMD NEFF for top-k = k (k <= 8)."""
    import concourse.bacc as bacc
    import concourse.mybir as mybir
    import concourse.tile as tile

    bf16 = mybir.dt.bfloat16
    fp8 = mybir.dt.float8e4
    f32 = mybir.dt.float32

    topk = _paged_topk_sum_op(k)

    nc = bacc.Bacc("TRN2", target_bir_lowering=False, debug=False)
    # per-episode packed layouts: one contiguous DMA per episode
    qp8 = nc.dram_tensor("qp8", [EPC, 128, ND * QP_PAD], fp8, kind="ExternalInput")
    q0h = nc.dram_tensor("q0h", [128, ND * HEAD], fp8, kind="ExternalInput")
    q0r = nc.dram_tensor("q0r", [128, ND * (QP_PAD - HEAD)], fp8,
                         kind="ExternalInput")
    seh = nc.dram_tensor("seh", [EPC, 128, ND * SCOLS], bf16, kind="ExternalInput")
    # per-episode top-k sums per (query-patch, class); host does the
    # patch->query aggregation (0.3% of the FLOPs)
    draw = nc.dram_tensor("draw", [EPC, 128, NT * WAY], f32,
                          kind="ExternalOutput")

    with tile.TileContext(nc) as tc:
        with (
            tc.tile_pool(name="const", bufs=1) as cpool,
            tc.tile_pool(name="qe0", bufs=1) as e0pool,
            tc.tile_pool(name="q", bufs=3) as qpool,
            tc.tile_pool(name="simps", bufs=7, space="PSUM") as simpool,
            tc.tile_pool(name="draw", bufs=3) as drpool,
        ):
            # ALL input DMAs ride ONE queue in exact consumption order.
            seh_t = {0: cpool.tile([128, ND * SCOLS], bf16, tag="seh0",
                                   name="seh0")}
            nc.sync.dma_start(seh_t[0][:], seh[0])
            q0h_t = cpool.tile([128, ND * HEAD], fp8)
            nc.sync.dma_start(q0h_t[:], q0h[:])
            # rest of episode 0 in two host-packed contiguous chunk tiles
            qe0b = e0pool.tile([128, ND * (C1 - HEAD)], fp8)
            nc.sync.dma_start(qe0b[:], q0r[:, 0:ND * (C1 - HEAD)])
            qe0c = e0pool.tile([128, ND * (QP_PAD - C1)], fp8)
            nc.sync.dma_start(qe0c[:], q0r[:, ND * (C1 - HEAD):])

            # Brief PE warm-up while episode 0's head DMA lands.  The PE is
            # the continuously-busy bottleneck engine now, so the HAM clock
            # ramp (1.2 -> 2.4 GHz after ~3.4us sustained) completes during
            # the first real matmuls; burning the full ramp in dummies would
            # delay episode 0 by more than the cold-clock tax costs.
            wt = cpool.tile([128, 500], bf16)
            nc.gpsimd.memset(wt[:], 0.0)
            wps = simpool.tile([128, 500], f32, tag="simps", name="warmps")
            for _ in range(4):
                nc.tensor.matmul(wps[:], wt[:, 0:128], wt[:],
                                 start=True, stop=True)

            for e in range(EPC):
                if e == 0:
                    def qsl(t, d):
                        c = t * 128
                        if c < HEAD:
                            return q0h_t[:, d * HEAD + c:d * HEAD + c + 128]
                        if c < C1:
                            w = C1 - HEAD
                            return qe0b[:, d * w + c - HEAD:d * w + c - HEAD + 128]
                        w = QP_PAD - C1
                        return qe0c[:, d * w + c - C1:d * w + c - C1 + 128]
                else:
                    # prefetch in consumption order on the single DMA queue
                    seh_t[e] = cpool.tile([128, ND * SCOLS], bf16,
                                          tag=f"seh{e}", name=f"seh{e}")
                    nc.sync.dma_start(seh_t[e][:], seh[e])
                    qt = qpool.tile([128, ND * QP_PAD], fp8)
                    nc.sync.dma_start(qt[:], qp8[e])

                    def qsl(t, d, qt=qt):
                        return qt[:, d * QP_PAD + t * 128:d * QP_PAD + t * 128 + 128]

                def ssl(d, e=e):
                    return seh_t[e][:, d * SCOLS:(d + 1) * SCOLS]

                drt = drpool.tile([128, NT * WAY], f32)
                # group 0 covers exactly episode 0's head chunk (tiles 0-2),
                # so the first matmuls wait only on seh0 + q0h (~405KB)
                groups = [(0, 3), (3, 7), (7, 11), (11, 15)]
                for t0, t1 in groups:
                    w = (t1 - t0) * SCOLS
                    simps = simpool.tile([128, 500], f32, tag="simps")
                    for t in range(t0, t1):
                        off = (t - t0) * SCOLS
                        for d in range(ND):
                            nc.tensor.matmul(
                                simps[:, off:off + SCOLS],
                                qsl(t, d), ssl(d),
                                start=(d == 0), stop=(d == ND - 1),
                            )
                    # paged top-k-sum straight from PSUM: one value per
                    # (qp-tile, class) page of 25 support patches
                    nc.vector._custom_dve(
                        topk,
                        out=drt[:, t0 * WAY:t1 * WAY],
                        in0=simps[:, 0:w].rearrange("p (g n) -> p g n", n=HW),
                    )
                nc.scalar.dma_start(draw[e], drt[:])
    nc.compile()
    return nc


def kernel(base, query, r, neighbor_k):
    from concourse.bass_utils import run_bass_kernel_spmd

    k = int(neighbor_k)
    assert 1 <= k <= 8, f"top-k must fit the Max8 swap chain, got {k}"
    base = np.asarray(base, dtype=np.float32).reshape(B, WAY, D, HW)
    query = np.asarray(query, dtype=np.float32).reshape(B, NQ, D, HW)
    r = np.asarray(r, dtype=np.float32)

    # ---- host prep (layout + normalization) ----
    # support: normalized columns packed per episode -> [B, 128, ND*125] bf16
    s_norm = base / np.linalg.norm(base, axis=2, keepdims=True)
    s_ext = s_norm.transpose(0, 2, 1, 3).reshape(B, ND, 128, SCOLS)
    seh = np.ascontiguousarray(s_ext.transpose(0, 2, 1, 3)).reshape(
        B, 128, ND * SCOLS).astype(ml_dtypes.bfloat16)

    # query: normalized patches scaled x16 in fp8, packed [B, 128, ND*1920]
    qn = np.sqrt(np.einsum("bqdp,bqdp->bqp", query, query))      # [B, nq, hw]
    q_hat = query * (QSCALE / qn[:, :, None, :])
    q_mat = np.zeros((B, D, QP_PAD), dtype=ml_dtypes.float8_e4m3)
    q_mat[:, :, :QP] = q_hat.transpose(0, 2, 1, 3).reshape(B, D, QP)
    qp8 = np.ascontiguousarray(
        q_mat.reshape(B, ND, 128, QP_PAD).transpose(0, 2, 1, 3)).reshape(
        B, 128, ND * QP_PAD)

    # global-cosine branch on host (0.3% of the FLOPs)
    bmean = base.mean(axis=3)                                     # [B, way, D]
    bm = bmean / np.maximum(
        np.linalg.norm(bmean, axis=2, keepdims=True), GEPS)
    qmean = query.mean(axis=3)                                    # [B, nq, D]
    qm_hat = qmean / np.maximum(
        np.linalg.norm(qmean, axis=2, keepdims=True), GEPS)
    glob = np.einsum("bqd,bkd->bqk", qm_hat, bm)                  # [B, nq, way]

    # patch->query aggregation matrix (0/1), [128, NT, NQ] (host-side einsum)
    am = np.zeros((128, NT, NQ), dtype=np.float32)
    for t in range(NT):
        qp_idx = t * 128 + np.arange(128)
        valid = qp_idx < QP
        am[valid, t, qp_idx[valid] // HW] = 1.0

    if k not in _CACHE:
        _CACHE[k] = _build(k)
    nc = _CACHE[k]

    in_maps = []
    for c in range(N_CORES):
        sl = slice(c * EPC, (c + 1) * EPC)
        qc = qp8[sl]                                              # [EPC,128,ND*1920]
        in_maps.append({
            "qp8": qc,
            "q0h": np.ascontiguousarray(
                qc[0].reshape(128, ND, QP_PAD)[:, :, :HEAD]).reshape(
                128, ND * HEAD),
            "q0r": np.concatenate([
                np.ascontiguousarray(
                    qc[0].reshape(128, ND, QP_PAD)[:, :, HEAD:C1]).reshape(
                    128, ND * (C1 - HEAD)),
                np.ascontiguousarray(
                    qc[0].reshape(128, ND, QP_PAD)[:, :, C1:]).reshape(
                    128, ND * (QP_PAD - C1))], axis=1),
            "seh": seh[sl],
        })
    global _LAST_IN_MAPS
    _LAST_IN_MAPS = in_maps
    res = run_bass_kernel_spmd(nc, in_maps, list(range(N_CORES)))
    dn4 = np.empty((N_CORES, EPC, WAY, NQ), dtype=np.float32)
    for c in range(N_CORES):
        dr = np.asarray(res.results[c]["draw"], dtype=np.float32)  # [EPC,128,75]
        dn4[c] = np.einsum(
            "ptq,eptw->ewq", am, dr.reshape(EPC, 128, NT, WAY))
    dn4 = dn4.reshape(B, WAY, NQ).transpose(0, 2, 1) / (QSCALE * k)  # [B, nq, way]
    return (r[0] * glob + r[1] * dn4).astype(np.float32)
